# revision 8
# baseline (speedup 1.0000x reference)
"""AVSL similarity kernel for Trainium2 (8 NeuronCores, data-parallel over B1).

Math (per (b1,b2) pair, d-vector chain over 3 layers):
  n_l = (normalize(emb1_l[b1]) - normalize(emb2_l[b2]))**2        [D]
  hat_0 = n_0
  hat_l = (1-P_l) * (hat_{l-1} @ W_l) + P_l * n_l,  l=1,2
  P_l   = sigmoid(alpha_l * cert1_l[b1] * cert2_l[b2] + beta_l)
  W_l   = col-top3-masked, col-normalized link_{l-1}
  out[b1,b2] = sum_d hat_2

Device decomposition, [d(=128 partitions), b2(=512 free)] layout, Q_l = 1-P_l
(sigmoid of negated argument):
  A  = n1 - W1^T n0                       (PE: negV0 matmul + I*negE0 + I*n1)
  v1 = Q1 * A          => hat1 = n1 - v1  (DVE STT; Q via ACT)
  B  = n2 - W2^T n1 + W2^T v1             (PE)
  v2 = Q2 * B          => hat2 = n2 - v2  (DVE)
  out_row = (2 - 2*E1n2.E2n2^T)[r,:] - 1^T v2
            (closed-form sum_d n2; S2 rows land in C4 via a PE identity
             matmul, colsum matmuls accumulate -1^T v2 on top, and the
             result is DMA'd to DRAM directly from PSUM.)
Rows processed in pairs; n1/n2/Q tiles span the pair ([128,1024]) so the
shared-stationary matmuls run at FD=1024 (halves the PE instruction count).
Engine split for the n1/n2 squares is tunable per (r%16) slot across
ACT (Square activation), DVE (bf16 TS add + TT mul) and GPSIMD.
Matmul operands bf16.

Sharding: emb1/cert1 rows split 64/core; emb2/cert2/links/alpha/beta replicated.
"""
import os
import sys

sys.path.insert(0, "/opt/trn_rl_repo")

import numpy as np

import concourse.bass as bass
import concourse.tile as tile
from concourse import bacc, mybir
from concourse.bass_utils import run_bass_kernel_spmd

N_CORES = 8
B1, B2, D = 512, 512, 128
RPC = B1 // N_CORES  # rows of ovr_sim per core
F32 = mybir.dt.float32
BF16 = mybir.dt.bfloat16
AF = mybir.ActivationFunctionType
OP = mybir.AluOpType
AX = mybir.AxisListType

# per (r % 16) engine assignment for the n1/n2 elementwise passes:
# 'A' = ACT (Square activation), 'D' = DVE (bf16 TS add + TT mul),
# 'G' = GPSIMD (broadcast add + mul) — tuned so ACT/DVE/GPS busy balance
N1_ENG = ['G' if s % 4 == 1 else ('D' if s % 4 == 3 else 'A') for s in range(16)]
N2_ENG = ['G' if s % 4 == 3 else ('A' if s in (5, 13) else 'D') for s in range(16)]


def _bcast(ap_col, cols):
    return ap_col.broadcast_to((128, cols))

_cache = {}


def _norm_blocks(nc, pre, t, tag_prefix, parts):
    """l2-normalize rows of an SBUF tile [parts,128]; returns normalized tile."""
    sq = pre.tile([parts, 128], F32, tag=f"{tag_prefix}sq")
    nc.vector.tensor_mul(sq[:], t[:], t[:])
    ss = pre.tile([parts, 1], F32, tag=f"{tag_prefix}ss")
    nc.vector.reduce_sum(ss[:], sq[:], axis=AX.X)
    nrm = pre.tile([parts, 1], F32, tag=f"{tag_prefix}nrm")
    nc.scalar.sqrt(nrm[:], ss[:])
    nrm2 = pre.tile([parts, 1], F32, tag=f"{tag_prefix}nrm2")
    nc.vector.tensor_scalar_max(nrm2[:], nrm[:], 1e-12)
    rn = pre.tile([parts, 1], F32, tag=f"{tag_prefix}rn")
    nc.vector.reciprocal(rn[:], nrm2[:])
    tn = pre.tile([parts, 128], F32, tag=f"{tag_prefix}tn")
    nc.vector.tensor_scalar_mul(tn[:], t[:], rn[:])
    return tn


def _transpose_512(nc, pre, pps_tile, ident, dram_ap, normalize):
    """Load [512,128] DRAM tensor, optionally l2-normalize rows, transpose
    into the given PSUM tile [128,512]."""
    for blk in range(4):
        t = pre.tile([128, 128], F32, tag="ld")
        nc.sync.dma_start(t[:], dram_ap[blk * 128 : (blk + 1) * 128, :])
        if normalize:
            t = _norm_blocks(nc, pre, t, "n", 128)
        nc.tensor.transpose(pps_tile[:, blk * 128 : (blk + 1) * 128], t[:], ident[:])


def _prep_link(nc, pre, pps, const, ident, dram_ap, i, want_pos, want_f32=False):
    """Top-3-per-column mask + column-normalize of link [d,e].
    Returns (negW bf16 [d,e], W bf16 [d,e] or None, negW fp32 or None)."""
    lt = pre.tile([128, 128], F32, tag="wld")
    nc.sync.dma_start(lt[:], dram_ap[:, :])
    tpw = pps.tile([128, 128], F32, tag="tpw")
    nc.tensor.transpose(tpw[:], lt[:], ident[:])
    wt = pre.tile([128, 128], F32, tag="wt")
    nc.scalar.copy(wt[:], tpw[:])  # [e, d]

    x = wt
    m = None
    for k in range(3):
        m = pre.tile([128, 1], F32, tag=f"wm{k}")
        nc.vector.reduce_max(m[:], x[:], axis=AX.X)
        if k < 2:
            msk = pre.tile([128, 128], F32, tag=f"wmask{k}")
            # ((x >= m) * -2) + x : push current max below everything
            nc.vector.tensor_scalar(msk[:], x[:], m[:], -2.0, op0=OP.is_ge, op1=OP.mult)
            x2 = pre.tile([128, 128], F32, tag=f"wx{k}")
            nc.vector.tensor_add(x2[:], x[:], msk[:])
            x = x2
    # m = 3rd-largest original value per row; keep entries >= m
    wm = pre.tile([128, 128], F32, tag="wkeep")
    nc.vector.scalar_tensor_tensor(wm[:], wt[:], m[:], wt[:], op0=OP.is_ge, op1=OP.mult)
    cs = pre.tile([128, 1], F32, tag="wcs")
    nc.vector.reduce_sum(cs[:], wm[:], axis=AX.X)
    cse = pre.tile([128, 1], F32, tag="wcse")
    nc.vector.tensor_scalar_add(cse[:], cs[:], 1e-8)
    rc = pre.tile([128, 1], F32, tag="wrc")
    nc.vector.reciprocal(rc[:], cse[:])
    nrc = pre.tile([128, 1], F32, tag="wnrc")
    nc.scalar.mul(nrc[:], rc[:], -1.0)
    wnT = pre.tile([128, 128], F32, tag=f"wnT{i}", name=f"wnT{i}")
    nc.vector.tensor_scalar_mul(wnT[:], wm[:], nrc[:])  # [e, d] (negated)
    tpw2 = pps.tile([128, 128], F32, tag="tpw")
    nc.tensor.transpose(tpw2[:], wnT[:], ident[:])
    negw = const.tile([128, 128], BF16, tag=f"negW{i}", name=f"negW{i}")
    nc.scalar.copy(negw[:], tpw2[:])  # [d, e] bf16, negated
    posw = None
    if want_pos:
        posw = const.tile([128, 128], BF16, tag=f"posW{i}", name=f"posW{i}")
        nc.scalar.mul(posw[:], tpw2[:], -1.0)  # [d, e] bf16, positive
    negwf = None
    if want_f32:
        negwf = const.tile([128, 128], F32, tag=f"negWf{i}", name=f"negWf{i}")
        nc.scalar.copy(negwf[:], tpw2[:])  # [d, e] fp32, negated
    return negw, posw, negwf


def _build():
    nc = bacc.Bacc("TRN2", target_bir_lowering=False, debug=False)
    de1 = [nc.dram_tensor(f"emb1_{l}", [RPC, D], F32, kind="ExternalInput") for l in range(3)]
    dc1 = [nc.dram_tensor(f"cert1_{l}", [RPC, D], F32, kind="ExternalInput") for l in (1, 2)]
    de2 = [nc.dram_tensor(f"emb2_{l}", [B2, D], F32, kind="ExternalInput") for l in range(3)]
    dc2 = [nc.dram_tensor(f"cert2_{l}", [B2, D], F32, kind="ExternalInput") for l in (1, 2)]
    dal = [nc.dram_tensor(f"alpha_{l}", [D, 1], F32, kind="ExternalInput") for l in (1, 2)]
    dbe = [nc.dram_tensor(f"beta_{l}", [D, 1], F32, kind="ExternalInput") for l in (1, 2)]
    dlk = [nc.dram_tensor(f"link_{l}", [D, D], F32, kind="ExternalInput") for l in range(2)]
    did = nc.dram_tensor("ident", [D, D], F32, kind="ExternalInput")
    dout = nc.dram_tensor("ovr", [RPC, B2], F32, kind="ExternalOutput")

    with tile.TileContext(nc) as tc:
        with tc.tile_pool(name="const", bufs=1) as const:
            ident = const.tile([128, 128], F32, tag="ident")
            nc.sync.dma_start(ident[:], did.ap())
            identb = const.tile([128, 128], BF16, tag="identb")
            nc.vector.tensor_copy(identb[:], ident[:])
            onesb = const.tile([128, 1], BF16, tag="onesb")
            nc.vector.memset(onesb[:], 1.0)
            negonesb = const.tile([128, 1], BF16, tag="negonesb")
            nc.vector.memset(negonesb[:], -1.0)
            nacol = []
            nbcol = []
            for i in range(2):
                a = const.tile([128, 1], F32, tag=f"acol{i}", name=f"acol{i}")
                nc.sync.dma_start(a[:], dal[i].ap())
                na = const.tile([128, 1], F32, tag=f"nacol{i}", name=f"nacol{i}")
                nc.scalar.mul(na[:], a[:], -1.0)
                nacol.append(na)
                b = const.tile([128, 1], F32, tag=f"bcol{i}", name=f"bcol{i}")
                nc.sync.dma_start(b[:], dbe[i].ap())
                nb = const.tile([128, 1], F32, tag=f"nbcol{i}", name=f"nbcol{i}")
                nc.scalar.mul(nb[:], b[:], -1.0)
                nbcol.append(nb)

            # e2T: l=1/2 fp32 (ACT in) + bf16 (DVE/GPS in); l=0 bf16 (matmul rhs)
            e2T = [None] * 3
            e2T1b = None
            e2T2b = None
            c2T = [None] * 2
            e1T = [None] * 3  # l=0,2: positive; l=1: negated (ACT bias)
            ne1T2 = None  # negated l=2 (ACT bias for ACT-rows)
            nscT = [None] * 2
            with tc.tile_pool(name="pre", bufs=4) as pre, tc.tile_pool(
                name="prepsum", bufs=2, space="PSUM"
            ) as pps:
                for l in range(3):
                    tp = pps.tile([128, 512], F32, tag="tp512")
                    _transpose_512(nc, pre, tp, ident, de2[l].ap(), True)
                    dt = F32 if l in (1, 2) else BF16
                    e2T[l] = const.tile([128, 512], dt, tag=f"e2T{l}", name=f"e2T{l}")
                    nc.scalar.copy(e2T[l][:], tp[:])
                    if l == 1:
                        e2T1b = const.tile([128, 512], BF16, tag="e2T1b")
                        nc.vector.tensor_copy(e2T1b[:], tp[:])
                    if l == 2:
                        e2T2b = const.tile([128, 512], BF16, tag="e2T2b")
                        nc.vector.tensor_copy(e2T2b[:], tp[:])
                for i in range(2):
                    tp = pps.tile([128, 512], F32, tag="tp512")
                    _transpose_512(nc, pre, tp, ident, dc2[i].ap(), False)
                    c2T[i] = const.tile([128, 512], BF16, tag=f"c2T{i}", name=f"c2T{i}")
                    nc.scalar.copy(c2T[i][:], tp[:])
                # emb1 shard: normalize rows, transpose -> [d, r]
                for l in range(3):
                    t = pre.tile([64, 128], F32, tag="e1ld")
                    nc.sync.dma_start(t[:], de1[l].ap())
                    tn = _norm_blocks(nc, pre, t, "e1", 64)
                    if l == 1:
                        tn2 = pre.tile([64, 128], F32, tag="e1neg")
                        nc.scalar.mul(tn2[:], tn[:], -1.0)
                        tn = tn2
                    tp64 = pps.tile([128, 64], F32, tag="tp64")
                    nc.tensor.transpose(tp64[:], tn[:], ident[:64, :64])
                    e1T[l] = const.tile([128, 64], F32, tag=f"e1T{l}", name=f"e1T{l}")
                    nc.scalar.copy(e1T[l][:], tp64[:])
                    if l == 2:
                        ne1T2 = const.tile([128, 64], F32, tag="ne1T2")
                        nc.scalar.mul(ne1T2[:], tp64[:], -1.0)
                        m2e1T2b = const.tile([128, 64], BF16, tag="m2e1T2b")
                        nc.scalar.mul(m2e1T2b[:], tp64[:], -2.0)
                # cert1 shard: transpose, scale by -alpha -> [d, r]
                for i in range(2):
                    t = pre.tile([64, 128], F32, tag="c1ld")
                    nc.sync.dma_start(t[:], dc1[i].ap())
                    tp64 = pps.tile([128, 64], F32, tag="tp64")
                    nc.tensor.transpose(tp64[:], t[:], ident[:64, :64])
                    c1T = pre.tile([128, 64], F32, tag="c1T")
                    nc.scalar.copy(c1T[:], tp64[:])
                    nscT[i] = const.tile([128, 64], F32, tag=f"nscT{i}", name=f"nscT{i}")
                    nc.vector.tensor_scalar_mul(nscT[i][:], c1T[:], nacol[i][:])
                negW1, _, negW1f = _prep_link(nc, pre, pps, const, ident, dlk[0].ap(), 0, False, True)
                negW2, posW2, _ = _prep_link(nc, pre, pps, const, ident, dlk[1].ap(), 1, True)
                # constants for virtualized n0: n0 = e2sq0 + a0*e2T0 + c0
                e2sqT0 = pre.tile([128, 512], F32, tag="e2sqT0", name="e2sqT0")
                nc.vector.tensor_mul(e2sqT0[:], e2T[0][:], e2T[0][:])
                tpE = pps.tile([128, 512], F32, tag="tp512")
                nc.tensor.matmul(tpE[:], lhsT=negW1f[:], rhs=e2sqT0[:], start=True, stop=True)
                negE0 = const.tile([128, 512], BF16, tag="negE0", name="negE0")
                nc.scalar.copy(negE0[:], tpE[:])
                a0T = const.tile([128, 64], F32, tag="a0T", name="a0T")
                nc.scalar.mul(a0T[:], e1T[0][:], -2.0)
                c0T = pre.tile([128, 64], F32, tag="c0T", name="c0T")
                nc.vector.tensor_mul(c0T[:], e1T[0][:], e1T[0][:])
                tpD = pps.tile([128, 64], F32, tag="tp64")
                nc.tensor.matmul(tpD[:], lhsT=negW1f[:], rhs=c0T[:], start=True, stop=True)
                negd0T = const.tile([128, 64], F32, tag="negd0T", name="negd0T")
                nc.scalar.copy(negd0T[:], tpD[:])
                # bf16 helpers for TS scalars / GPSIMD row passes
                a0Tb = const.tile([128, 64], BF16, tag="a0Tb")
                nc.vector.tensor_copy(a0Tb[:], a0T[:])
                ne1T1b = const.tile([128, 64], BF16, tag="ne1T1b")
                nc.vector.tensor_copy(ne1T1b[:], e1T[1][:])  # e1T[1] is negated
                ne1T2b = const.tile([128, 64], BF16, tag="ne1T2b")
                nc.vector.tensor_copy(ne1T2b[:], ne1T2[:])
                # closed-form row-sum of n2: sum_d n2 = 2 - 2*E1n2.E2n2^T
                psS = pps.tile([64, 512], F32, tag="tpS")
                nc.tensor.matmul(psS[:], lhsT=m2e1T2b[:], rhs=e2T2b[:], start=True, stop=True)
                twos = const.tile([64, 1], F32, tag="twos")
                nc.vector.memset(twos[:], 2.0)
                S2sb = const.tile([64, 512], BF16, tag="S2sb", name="S2sb")
                nc.scalar.activation(S2sb[:], psS[:], AF.Identity, bias=twos[:])
                # rearrange S2 rows to the C4 partition layout (row 4g+k ->
                # partition 32k, free block g); zero unused partitions so the
                # identity matmul into C4 reads defined data everywhere
                S2str = const.tile([128, 8192], BF16, tag="S2str", name="S2str")
                nc.vector.memset(S2str[:], 0.0)
                for k in range(4):
                    nc.sync.dma_start(
                        S2str[:][32 * k : 32 * k + 1, :], S2sb[:][k:64:4, :]
                    )

            with tc.tile_pool(name="row", bufs=10) as rowp, tc.tile_pool(
                name="pair", bufs=4
            ) as pairp, tc.tile_pool(name="psA", bufs=2, space="PSUM") as psA, tc.tile_pool(
                name="psB", bufs=1, space="PSUM"
            ) as psB, tc.tile_pool(name="psC", bufs=2, space="PSUM") as psC:
                C4 = None
                for rp in range(RPC // 2):
                    r0 = 2 * rp
                    Ap = psA.tile([128, 1024], F32, tag="Ap")
                    Bpair = psB.tile([128, 1024], F32, tag="Bpair")
                    Q1p = pairp.tile([128, 1024], F32, tag="Q1p")
                    Q2p = pairp.tile([128, 1024], F32, tag="Q2p")
                    n1p = pairp.tile([128, 1024], BF16, tag="n1p")
                    n2p = pairp.tile([128, 1024], BF16, tag="n2p")
                    n0h = [None, None]
                    v1p = pairp.tile([128, 1024], BF16, tag="v1p")
                    v2p = pairp.tile([128, 1024], BF16, tag="v2p")
                    if rp % 2 == 0:
                        # fresh C4: seed with the S2 closed-form rows via PE
                        C4 = psC.tile([128, 512], F32, tag="C4")
                        g4 = rp // 2
                        nc.tensor.matmul(
                            C4[:, :], lhsT=identb[:],
                            rhs=S2str[:, 512 * g4 : 512 * g4 + 512],
                            start=True, stop=False,
                        )
                    for h in range(2):
                        r = r0 + h
                        fo = 512 * h
                        # n0 virtualized: negV0 = a0 * (-W1); its matmul against
                        # e2T0 plus I*negE0 reproduce -W1^T n0 up to the
                        # per-partition constant negd0T handled in the v1 STT
                        negV0 = rowp.tile([128, 128], BF16, tag="negV0")
                        nc.vector.tensor_scalar_mul(
                            negV0[:], negW1[:], a0T[:, r : r + 1]
                        )
                        n0h[h] = negV0
                        # n1 on ACT / DVE / GPS per row slot
                        e1 = N1_ENG[r % 16]
                        if e1 == "A":
                            nc.scalar.activation(
                                n1p[:, fo : fo + 512], e2T[1][:], AF.Square,
                                bias=e1T[1][:, r : r + 1],
                            )
                        elif e1 == "D":
                            d1 = rowp.tile([128, 512], BF16, tag="d1")
                            nc.vector.tensor_scalar_add(
                                d1[:], e2T1b[:], e1T[1][:, r : r + 1]
                            )
                            nc.vector.tensor_mul(n1p[:, fo : fo + 512], d1[:], d1[:])
                        else:
                            d1 = rowp.tile([128, 512], BF16, tag="d1")
                            nc.gpsimd.tensor_tensor(
                                d1[:], e2T1b[:], _bcast(ne1T1b[:, r : r + 1], 512),
                                op=OP.add,
                            )
                            nc.gpsimd.tensor_mul(n1p[:, fo : fo + 512], d1[:], d1[:])
                        # n2 on ACT / DVE / GPS per row slot
                        e2 = N2_ENG[r % 16]
                        if e2 == "D":
                            d2 = rowp.tile([128, 512], BF16, tag="d2")
                            nc.vector.tensor_scalar_add(
                                d2[:], e2T2b[:], ne1T2[:, r : r + 1]
                            )
                            nc.vector.tensor_mul(n2p[:, fo : fo + 512], d2[:], d2[:])
                        elif e2 == "A":
                            nc.scalar.activation(
                                n2p[:, fo : fo + 512], e2T[2][:], AF.Square,
                                bias=ne1T2[:, r : r + 1],
                            )
                        else:
                            d2 = rowp.tile([128, 512], BF16, tag="d2")
                            nc.gpsimd.tensor_tensor(
                                d2[:], e2T2b[:], _bcast(ne1T2b[:, r : r + 1], 512),
                                op=OP.add,
                            )
                            nc.gpsimd.tensor_mul(n2p[:, fo : fo + 512], d2[:], d2[:])
                        nc.scalar.activation(
                            Q1p[:, fo : fo + 512],
                            c2T[0][:],
                            AF.Sigmoid,
                            bias=nbcol[0][:],
                            scale=nscT[0][:, r : r + 1],
                        )
                        nc.scalar.activation(
                            Q2p[:, fo : fo + 512],
                            c2T[1][:],
                            AF.Sigmoid,
                            bias=nbcol[1][:],
                            scale=nscT[1][:, r : r + 1],
                        )
                    # matmuls interleaved across the two halves so consecutive
                    # PE ops never accumulate into the same PSUM region
                    for h in range(2):
                        nc.tensor.matmul(
                            Ap[:, 512 * h : 512 * h + 512], lhsT=n0h[h][:],
                            rhs=e2T[0][:], start=True, stop=False,
                        )
                    for h in range(2):
                        nc.tensor.matmul(
                            Ap[:, 512 * h : 512 * h + 512], lhsT=identb[:],
                            rhs=negE0[:], start=False, stop=False,
                        )
                    for h in range(2):
                        nc.tensor.matmul(
                            Ap[:, 512 * h : 512 * h + 512], lhsT=identb[:],
                            rhs=n1p[:, 512 * h : 512 * h + 512], start=False, stop=True,
                        )
                    for h in range(2):
                        nc.tensor.matmul(
                            Bpair[:, 512 * h : 512 * h + 512], lhsT=negW2[:],
                            rhs=n1p[:, 512 * h : 512 * h + 512], start=True, stop=False,
                        )
                    # v1 = Q1 * (A + negd0) per half (STT: scalar slot carries
                    # the per-partition constant term of -W1^T n0)
                    for h in range(2):
                        r = r0 + h
                        nc.vector.scalar_tensor_tensor(
                            v1p[:, 512 * h : 512 * h + 512],
                            Ap[:, 512 * h : 512 * h + 512],
                            negd0T[:, r : r + 1],
                            Q1p[:, 512 * h : 512 * h + 512],
                            op0=OP.add,
                            op1=OP.mult,
                        )
                    for h in range(2):
                        nc.tensor.matmul(
                            Bpair[:, 512 * h : 512 * h + 512], lhsT=posW2[:],
                            rhs=v1p[:, 512 * h : 512 * h + 512], start=False, stop=False,
                        )
                    for h in range(2):
                        nc.tensor.matmul(
                            Bpair[:, 512 * h : 512 * h + 512], lhsT=identb[:],
                            rhs=n2p[:, 512 * h : 512 * h + 512], start=False, stop=True,
                        )
                    # batched v2 = Q2 * B over the pair
                    nc.vector.tensor_mul(v2p[:], Q2p[:], Bpair[:])
                    for h in range(2):
                        r = r0 + h
                        po = 32 * (r % 4)
                        nc.tensor.matmul(
                            C4[po : po + 1, :], lhsT=negonesb[:],
                            rhs=v2p[:, 512 * h : 512 * h + 512], start=False, stop=True,
                            tile_position=(0, po),
                        )
                    if rp % 2 == 1:
                        # C4 holds S2 - 1^T v2 = out rows; cheap fp32 copy to
                        # SBUF (DMA cannot read PSUM), then one DMA out
                        stag = rowp.tile([128, 512], F32, tag="stag")
                        nc.vector.tensor_copy(stag[:], C4[:])
                        nc.sync.dma_start(
                            dout.ap()[r0 - 2 : r0 + 2, :], stag[:][0:97:32, :]
                        )
    nc.compile()
    return nc


def _get_nc():
    if "nc" not in _cache:
        _cache["nc"] = _build()
    return _cache["nc"]


def kernel(**inputs):
    nc = _get_nc()
    ident = np.eye(D, dtype=np.float32)
    in_maps = []
    for c in range(N_CORES):
        sl = slice(c * RPC, (c + 1) * RPC)
        m = {"ident": ident}
        for l in range(3):
            m[f"emb1_{l}"] = np.ascontiguousarray(inputs[f"emb1_{l}"][sl])
            m[f"emb2_{l}"] = np.asarray(inputs[f"emb2_{l}"])
        for l in (1, 2):
            m[f"cert1_{l}"] = np.ascontiguousarray(inputs[f"cert1_{l}"][sl])
            m[f"cert2_{l}"] = np.asarray(inputs[f"cert2_{l}"])
            m[f"alpha_{l}"] = np.asarray(inputs[f"alpha_{l}"]).reshape(D, 1)
            m[f"beta_{l}"] = np.asarray(inputs[f"beta_{l}"]).reshape(D, 1)
        for l in range(2):
            m[f"link_{l}"] = np.asarray(inputs[f"link_{l}"])
        in_maps.append(m)
    trace = bool(int(os.environ.get("AVSL_TRACE", "0")))
    res = run_bass_kernel_spmd(nc, in_maps, core_ids=list(range(N_CORES)), trace=trace)
    _cache["last_result"] = res
    return np.concatenate([res.results[c]["ovr"] for c in range(N_CORES)], axis=0)


# revision 9
# speedup vs baseline: 1.0065x; 1.0065x over previous
"""AVSL similarity kernel for Trainium2 (8 NeuronCores, data-parallel over B1).

Math (per (b1,b2) pair, d-vector chain over 3 layers):
  n_l = (normalize(emb1_l[b1]) - normalize(emb2_l[b2]))**2        [D]
  hat_0 = n_0
  hat_l = (1-P_l) * (hat_{l-1} @ W_l) + P_l * n_l,  l=1,2
  P_l   = sigmoid(alpha_l * cert1_l[b1] * cert2_l[b2] + beta_l)
  W_l   = col-top3-masked, col-normalized link_{l-1}
  out[b1,b2] = sum_d hat_2

Device decomposition, [d(=128 partitions), b2(=512 free)] layout, Q_l = 1-P_l
(sigmoid of negated argument):
  A  = n1 - W1^T n0                       (PE: negV0 matmul + I*negE0 + I*n1)
  v1 = Q1 * A          => hat1 = n1 - v1  (DVE STT; Q via ACT)
  B  = n2 - W2^T n1 + W2^T v1             (PE)
  v2 = Q2 * B          => hat2 = n2 - v2  (DVE)
  out_row = (2 - 2*E1n2.E2n2^T)[r,:] - 1^T v2
            (closed-form sum_d n2; S2 rows land in C4 via a PE identity
             matmul, colsum matmuls accumulate -1^T v2 on top, and the
             result is DMA'd to DRAM directly from PSUM.)
Rows processed in pairs; n1/n2/Q tiles span the pair ([128,1024]) so the
shared-stationary matmuls run at FD=1024 (halves the PE instruction count).
Engine split for the n1/n2 squares is tunable per (r%16) slot across
ACT (Square activation), DVE (bf16 TS add + TT mul) and GPSIMD.
Matmul operands bf16.

Sharding: emb1/cert1 rows split 64/core; emb2/cert2/links/alpha/beta replicated.
"""
import os
import sys

sys.path.insert(0, "/opt/trn_rl_repo")

import numpy as np

import concourse.bass as bass
import concourse.tile as tile
from concourse import bacc, mybir
from concourse.bass_utils import run_bass_kernel_spmd

N_CORES = 8
B1, B2, D = 512, 512, 128
RPC = B1 // N_CORES  # rows of ovr_sim per core
F32 = mybir.dt.float32
BF16 = mybir.dt.bfloat16
AF = mybir.ActivationFunctionType
OP = mybir.AluOpType
AX = mybir.AxisListType

# per (r % 16) engine assignment for the n1/n2 elementwise passes:
# 'A' = ACT (Square activation, includes the E1^2 rank-1 term),
# 'D' = DVE (single STT: n = E2^2 - 2*E1*E2; the missing E1^2 term is
#       folded into the v1/v2 STT scalar columns, chosen per slot).
# GPSIMD was tried and is a net loss: SBUF port contention slows DVE
# ~20-25% and its semaphore handling costs ~0.9us per sync.
N1_ENG = ['A' if (s % 4 == 0 or s in (2, 6)) else 'D' for s in range(16)]
N2_ENG = ['A' if s in (1, 3, 5, 9, 13) else 'D' for s in range(16)]


def _bcast(ap_col, cols):
    return ap_col.broadcast_to((128, cols))

_cache = {}


def _norm_blocks(nc, pre, t, tag_prefix, parts):
    """l2-normalize rows of an SBUF tile [parts,128]; returns normalized tile."""
    sq = pre.tile([parts, 128], F32, tag=f"{tag_prefix}sq")
    nc.vector.tensor_mul(sq[:], t[:], t[:])
    ss = pre.tile([parts, 1], F32, tag=f"{tag_prefix}ss")
    nc.vector.reduce_sum(ss[:], sq[:], axis=AX.X)
    nrm = pre.tile([parts, 1], F32, tag=f"{tag_prefix}nrm")
    nc.scalar.sqrt(nrm[:], ss[:])
    nrm2 = pre.tile([parts, 1], F32, tag=f"{tag_prefix}nrm2")
    nc.vector.tensor_scalar_max(nrm2[:], nrm[:], 1e-12)
    rn = pre.tile([parts, 1], F32, tag=f"{tag_prefix}rn")
    nc.vector.reciprocal(rn[:], nrm2[:])
    tn = pre.tile([parts, 128], F32, tag=f"{tag_prefix}tn")
    nc.vector.tensor_scalar_mul(tn[:], t[:], rn[:])
    return tn


def _transpose_512(nc, pre, pps_tile, ident, dram_ap, normalize):
    """Load [512,128] DRAM tensor, optionally l2-normalize rows, transpose
    into the given PSUM tile [128,512]."""
    for blk in range(4):
        t = pre.tile([128, 128], F32, tag="ld")
        nc.sync.dma_start(t[:], dram_ap[blk * 128 : (blk + 1) * 128, :])
        if normalize:
            t = _norm_blocks(nc, pre, t, "n", 128)
        nc.tensor.transpose(pps_tile[:, blk * 128 : (blk + 1) * 128], t[:], ident[:])


def _prep_link(nc, pre, pps, const, ident, dram_ap, i, want_pos, want_f32=False):
    """Top-3-per-column mask + column-normalize of link [d,e].
    Returns (negW bf16 [d,e], W bf16 [d,e] or None, negW fp32 or None)."""
    lt = pre.tile([128, 128], F32, tag="wld")
    nc.sync.dma_start(lt[:], dram_ap[:, :])
    tpw = pps.tile([128, 128], F32, tag="tpw")
    nc.tensor.transpose(tpw[:], lt[:], ident[:])
    wt = pre.tile([128, 128], F32, tag="wt")
    nc.scalar.copy(wt[:], tpw[:])  # [e, d]

    x = wt
    m = None
    for k in range(3):
        m = pre.tile([128, 1], F32, tag=f"wm{k}")
        nc.vector.reduce_max(m[:], x[:], axis=AX.X)
        if k < 2:
            msk = pre.tile([128, 128], F32, tag=f"wmask{k}")
            # ((x >= m) * -2) + x : push current max below everything
            nc.vector.tensor_scalar(msk[:], x[:], m[:], -2.0, op0=OP.is_ge, op1=OP.mult)
            x2 = pre.tile([128, 128], F32, tag=f"wx{k}")
            nc.vector.tensor_add(x2[:], x[:], msk[:])
            x = x2
    # m = 3rd-largest original value per row; keep entries >= m
    wm = pre.tile([128, 128], F32, tag="wkeep")
    nc.vector.scalar_tensor_tensor(wm[:], wt[:], m[:], wt[:], op0=OP.is_ge, op1=OP.mult)
    cs = pre.tile([128, 1], F32, tag="wcs")
    nc.vector.reduce_sum(cs[:], wm[:], axis=AX.X)
    cse = pre.tile([128, 1], F32, tag="wcse")
    nc.vector.tensor_scalar_add(cse[:], cs[:], 1e-8)
    rc = pre.tile([128, 1], F32, tag="wrc")
    nc.vector.reciprocal(rc[:], cse[:])
    nrc = pre.tile([128, 1], F32, tag="wnrc")
    nc.scalar.mul(nrc[:], rc[:], -1.0)
    wnT = pre.tile([128, 128], F32, tag=f"wnT{i}", name=f"wnT{i}")
    nc.vector.tensor_scalar_mul(wnT[:], wm[:], nrc[:])  # [e, d] (negated)
    tpw2 = pps.tile([128, 128], F32, tag="tpw")
    nc.tensor.transpose(tpw2[:], wnT[:], ident[:])
    negw = const.tile([128, 128], BF16, tag=f"negW{i}", name=f"negW{i}")
    nc.scalar.copy(negw[:], tpw2[:])  # [d, e] bf16, negated
    posw = None
    if want_pos:
        posw = const.tile([128, 128], BF16, tag=f"posW{i}", name=f"posW{i}")
        nc.scalar.mul(posw[:], tpw2[:], -1.0)  # [d, e] bf16, positive
    negwf = None
    if want_f32:
        negwf = const.tile([128, 128], F32, tag=f"negWf{i}", name=f"negWf{i}")
        nc.scalar.copy(negwf[:], tpw2[:])  # [d, e] fp32, negated
    return negw, posw, negwf


def _build():
    nc = bacc.Bacc("TRN2", target_bir_lowering=False, debug=False)
    de1 = [nc.dram_tensor(f"emb1_{l}", [RPC, D], F32, kind="ExternalInput") for l in range(3)]
    dc1 = [nc.dram_tensor(f"cert1_{l}", [RPC, D], F32, kind="ExternalInput") for l in (1, 2)]
    de2 = [nc.dram_tensor(f"emb2_{l}", [B2, D], F32, kind="ExternalInput") for l in range(3)]
    dc2 = [nc.dram_tensor(f"cert2_{l}", [B2, D], F32, kind="ExternalInput") for l in (1, 2)]
    dal = [nc.dram_tensor(f"alpha_{l}", [D, 1], F32, kind="ExternalInput") for l in (1, 2)]
    dbe = [nc.dram_tensor(f"beta_{l}", [D, 1], F32, kind="ExternalInput") for l in (1, 2)]
    dlk = [nc.dram_tensor(f"link_{l}", [D, D], F32, kind="ExternalInput") for l in range(2)]
    did = nc.dram_tensor("ident", [D, D], F32, kind="ExternalInput")
    dout = nc.dram_tensor("ovr", [RPC, B2], F32, kind="ExternalOutput")

    with tile.TileContext(nc) as tc:
        with tc.tile_pool(name="const", bufs=1) as const:
            ident = const.tile([128, 128], F32, tag="ident")
            nc.sync.dma_start(ident[:], did.ap())
            identb = const.tile([128, 128], BF16, tag="identb")
            nc.vector.tensor_copy(identb[:], ident[:])
            onesb = const.tile([128, 1], BF16, tag="onesb")
            nc.vector.memset(onesb[:], 1.0)
            negonesb = const.tile([128, 1], BF16, tag="negonesb")
            nc.vector.memset(negonesb[:], -1.0)
            nacol = []
            nbcol = []
            for i in range(2):
                a = const.tile([128, 1], F32, tag=f"acol{i}", name=f"acol{i}")
                nc.sync.dma_start(a[:], dal[i].ap())
                na = const.tile([128, 1], F32, tag=f"nacol{i}", name=f"nacol{i}")
                nc.scalar.mul(na[:], a[:], -1.0)
                nacol.append(na)
                b = const.tile([128, 1], F32, tag=f"bcol{i}", name=f"bcol{i}")
                nc.sync.dma_start(b[:], dbe[i].ap())
                nb = const.tile([128, 1], F32, tag=f"nbcol{i}", name=f"nbcol{i}")
                nc.scalar.mul(nb[:], b[:], -1.0)
                nbcol.append(nb)

            # e2T: l=1/2 fp32 (ACT in) + bf16 (DVE/GPS in); l=0 bf16 (matmul rhs)
            e2T = [None] * 3
            e2T1b = None
            e2T2b = None
            c2T = [None] * 2
            e1T = [None] * 3  # l=0,2: positive; l=1: negated (ACT bias)
            ne1T2 = None  # negated l=2 (ACT bias for ACT-rows)
            nscT = [None] * 2
            with tc.tile_pool(name="pre", bufs=4) as pre, tc.tile_pool(
                name="prepsum", bufs=2, space="PSUM"
            ) as pps:
                for l in range(3):
                    tp = pps.tile([128, 512], F32, tag="tp512")
                    _transpose_512(nc, pre, tp, ident, de2[l].ap(), True)
                    dt = F32 if l in (1, 2) else BF16
                    e2T[l] = const.tile([128, 512], dt, tag=f"e2T{l}", name=f"e2T{l}")
                    nc.scalar.copy(e2T[l][:], tp[:])
                    if l == 1:
                        e2T1b = const.tile([128, 512], BF16, tag="e2T1b")
                        nc.vector.tensor_copy(e2T1b[:], tp[:])
                    if l == 2:
                        e2T2b = const.tile([128, 512], BF16, tag="e2T2b")
                        nc.vector.tensor_copy(e2T2b[:], tp[:])
                for i in range(2):
                    tp = pps.tile([128, 512], F32, tag="tp512")
                    _transpose_512(nc, pre, tp, ident, dc2[i].ap(), False)
                    c2T[i] = const.tile([128, 512], BF16, tag=f"c2T{i}", name=f"c2T{i}")
                    nc.scalar.copy(c2T[i][:], tp[:])
                # emb1 shard: normalize rows, transpose -> [d, r]
                for l in range(3):
                    t = pre.tile([64, 128], F32, tag="e1ld")
                    nc.sync.dma_start(t[:], de1[l].ap())
                    tn = _norm_blocks(nc, pre, t, "e1", 64)
                    if l == 1:
                        tn2 = pre.tile([64, 128], F32, tag="e1neg")
                        nc.scalar.mul(tn2[:], tn[:], -1.0)
                        tn = tn2
                    tp64 = pps.tile([128, 64], F32, tag="tp64")
                    nc.tensor.transpose(tp64[:], tn[:], ident[:64, :64])
                    e1T[l] = const.tile([128, 64], F32, tag=f"e1T{l}", name=f"e1T{l}")
                    nc.scalar.copy(e1T[l][:], tp64[:])
                    if l == 2:
                        ne1T2 = const.tile([128, 64], F32, tag="ne1T2")
                        nc.scalar.mul(ne1T2[:], tp64[:], -1.0)
                        m2e1T2b = const.tile([128, 64], BF16, tag="m2e1T2b")
                        nc.scalar.mul(m2e1T2b[:], tp64[:], -2.0)
                # cert1 shard: transpose, scale by -alpha -> [d, r]
                for i in range(2):
                    t = pre.tile([64, 128], F32, tag="c1ld")
                    nc.sync.dma_start(t[:], dc1[i].ap())
                    tp64 = pps.tile([128, 64], F32, tag="tp64")
                    nc.tensor.transpose(tp64[:], t[:], ident[:64, :64])
                    c1T = pre.tile([128, 64], F32, tag="c1T")
                    nc.scalar.copy(c1T[:], tp64[:])
                    nscT[i] = const.tile([128, 64], F32, tag=f"nscT{i}", name=f"nscT{i}")
                    nc.vector.tensor_scalar_mul(nscT[i][:], c1T[:], nacol[i][:])
                negW1, _, negW1f = _prep_link(nc, pre, pps, const, ident, dlk[0].ap(), 0, False, True)
                negW2, posW2, negW2f = _prep_link(nc, pre, pps, const, ident, dlk[1].ap(), 1, True, True)
                # constants for virtualized n0: n0 = e2sq0 + a0*e2T0 + c0
                e2sqT0 = pre.tile([128, 512], F32, tag="e2sqT0", name="e2sqT0")
                nc.vector.tensor_mul(e2sqT0[:], e2T[0][:], e2T[0][:])
                tpE = pps.tile([128, 512], F32, tag="tp512")
                nc.tensor.matmul(tpE[:], lhsT=negW1f[:], rhs=e2sqT0[:], start=True, stop=True)
                negE0 = const.tile([128, 512], BF16, tag="negE0", name="negE0")
                nc.scalar.copy(negE0[:], tpE[:])
                a0T = const.tile([128, 64], F32, tag="a0T", name="a0T")
                nc.scalar.mul(a0T[:], e1T[0][:], -2.0)
                c0T = pre.tile([128, 64], F32, tag="c0T", name="c0T")
                nc.vector.tensor_mul(c0T[:], e1T[0][:], e1T[0][:])
                tpD = pps.tile([128, 64], F32, tag="tp64")
                nc.tensor.matmul(tpD[:], lhsT=negW1f[:], rhs=c0T[:], start=True, stop=True)
                negd0T = const.tile([128, 64], F32, tag="negd0T", name="negd0T")
                nc.scalar.copy(negd0T[:], tpD[:])
                # cross-trick constants for DVE 'D' rows:
                # n_l(D-row) = e2sq_lb + (-2 E1_l) * E2_l   (one STT)
                e2sqT1b = const.tile([128, 512], BF16, tag="e2sqT1b")
                nc.vector.tensor_mul(e2sqT1b[:], e2T1b[:], e2T1b[:])
                e2sqT2b = const.tile([128, 512], BF16, tag="e2sqT2b")
                nc.vector.tensor_mul(e2sqT2b[:], e2T2b[:], e2T2b[:])
                m2e1T1 = const.tile([128, 64], F32, tag="m2e1T1")
                nc.scalar.mul(m2e1T1[:], e1T[1][:], 2.0)  # e1T[1] negated
                m2e1T2 = const.tile([128, 64], F32, tag="m2e1T2")
                nc.scalar.mul(m2e1T2[:], e1T[2][:], -2.0)
                # correction tiles: cA = negd0 (+E1sq1 for n1-D rows);
                # cB = (-W2^T E1sq1 for n1-D rows) + (E1sq2 for n2-D rows)
                e1sq1T = pre.tile([128, 64], F32, tag="e1sq1T")
                nc.vector.tensor_mul(e1sq1T[:], e1T[1][:], e1T[1][:])
                e1sq2T = pre.tile([128, 64], F32, tag="e1sq2T")
                nc.vector.tensor_mul(e1sq2T[:], e1T[2][:], e1T[2][:])
                cA_D = const.tile([128, 64], F32, tag="cA_D")
                nc.vector.tensor_add(cA_D[:], negd0T[:], e1sq1T[:])
                tpW = pps.tile([128, 64], F32, tag="tp64")
                nc.tensor.matmul(tpW[:], lhsT=negW2f[:], rhs=e1sq1T[:], start=True, stop=True)
                cB_10 = const.tile([128, 64], F32, tag="cB_10")
                nc.scalar.copy(cB_10[:], tpW[:])
                cB_01 = const.tile([128, 64], F32, tag="cB_01")
                nc.vector.tensor_copy(cB_01[:], e1sq2T[:])
                cB_11 = const.tile([128, 64], F32, tag="cB_11")
                nc.vector.tensor_add(cB_11[:], cB_10[:], e1sq2T[:])
                # closed-form row-sum of n2: sum_d n2 = 2 - 2*E1n2.E2n2^T
                psS = pps.tile([64, 512], F32, tag="tpS")
                nc.tensor.matmul(psS[:], lhsT=m2e1T2b[:], rhs=e2T2b[:], start=True, stop=True)
                twos = const.tile([64, 1], F32, tag="twos")
                nc.vector.memset(twos[:], 2.0)
                S2sb = const.tile([64, 512], BF16, tag="S2sb", name="S2sb")
                nc.scalar.activation(S2sb[:], psS[:], AF.Identity, bias=twos[:])
                # rearrange S2 rows to the C4 partition layout (row 4g+k ->
                # partition 32k, free block g); zero unused partitions so the
                # identity matmul into C4 reads defined data everywhere
                S2str = const.tile([128, 8192], BF16, tag="S2str", name="S2str")
                nc.vector.memset(S2str[:], 0.0)
                for k in range(4):
                    nc.sync.dma_start(
                        S2str[:][32 * k : 32 * k + 1, :], S2sb[:][k:64:4, :]
                    )

            with tc.tile_pool(name="row", bufs=10) as rowp, tc.tile_pool(
                name="pair", bufs=4
            ) as pairp, tc.tile_pool(name="psA", bufs=2, space="PSUM") as psA, tc.tile_pool(
                name="psB", bufs=1, space="PSUM"
            ) as psB, tc.tile_pool(name="psC", bufs=2, space="PSUM") as psC:
                C4 = None
                for rp in range(RPC // 2):
                    r0 = 2 * rp
                    Ap = psA.tile([128, 1024], F32, tag="Ap")
                    Bpair = psB.tile([128, 1024], F32, tag="Bpair")
                    Q1p = pairp.tile([128, 1024], F32, tag="Q1p")
                    Q2p = pairp.tile([128, 1024], F32, tag="Q2p")
                    n1p = pairp.tile([128, 1024], BF16, tag="n1p")
                    n2p = pairp.tile([128, 1024], BF16, tag="n2p")
                    n0h = [None, None]
                    v1p = pairp.tile([128, 1024], BF16, tag="v1p")
                    v2p = pairp.tile([128, 1024], BF16, tag="v2p")
                    if rp % 2 == 0:
                        # fresh C4: seed with the S2 closed-form rows via PE
                        C4 = psC.tile([128, 512], F32, tag="C4")
                        g4 = rp // 2
                        nc.tensor.matmul(
                            C4[:, :], lhsT=identb[:],
                            rhs=S2str[:, 512 * g4 : 512 * g4 + 512],
                            start=True, stop=False,
                        )
                    for h in range(2):
                        r = r0 + h
                        fo = 512 * h
                        # n0 virtualized: negV0 = a0 * (-W1); its matmul against
                        # e2T0 plus I*negE0 reproduce -W1^T n0 up to the
                        # per-partition constant negd0T handled in the v1 STT
                        negV0 = rowp.tile([128, 128], BF16, tag="negV0")
                        nc.vector.tensor_scalar_mul(
                            negV0[:], negW1[:], a0T[:, r : r + 1]
                        )
                        n0h[h] = negV0
                        # n1 on ACT (full square) or DVE (cross-trick STT)
                        if N1_ENG[r % 16] == "A":
                            nc.scalar.activation(
                                n1p[:, fo : fo + 512], e2T[1][:], AF.Square,
                                bias=e1T[1][:, r : r + 1],
                            )
                        else:
                            nc.vector.scalar_tensor_tensor(
                                n1p[:, fo : fo + 512], e2T1b[:],
                                m2e1T1[:, r : r + 1], e2sqT1b[:],
                                op0=OP.mult, op1=OP.add,
                            )
                        # n2 on ACT or DVE likewise
                        if N2_ENG[r % 16] == "A":
                            nc.scalar.activation(
                                n2p[:, fo : fo + 512], e2T[2][:], AF.Square,
                                bias=ne1T2[:, r : r + 1],
                            )
                        else:
                            nc.vector.scalar_tensor_tensor(
                                n2p[:, fo : fo + 512], e2T2b[:],
                                m2e1T2[:, r : r + 1], e2sqT2b[:],
                                op0=OP.mult, op1=OP.add,
                            )
                        nc.scalar.activation(
                            Q1p[:, fo : fo + 512],
                            c2T[0][:],
                            AF.Sigmoid,
                            bias=nbcol[0][:],
                            scale=nscT[0][:, r : r + 1],
                        )
                        nc.scalar.activation(
                            Q2p[:, fo : fo + 512],
                            c2T[1][:],
                            AF.Sigmoid,
                            bias=nbcol[1][:],
                            scale=nscT[1][:, r : r + 1],
                        )
                    # matmuls interleaved across the two halves so consecutive
                    # PE ops never accumulate into the same PSUM region
                    for h in range(2):
                        nc.tensor.matmul(
                            Ap[:, 512 * h : 512 * h + 512], lhsT=n0h[h][:],
                            rhs=e2T[0][:], start=True, stop=False,
                        )
                    for h in range(2):
                        nc.tensor.matmul(
                            Ap[:, 512 * h : 512 * h + 512], lhsT=identb[:],
                            rhs=negE0[:], start=False, stop=False,
                        )
                    for h in range(2):
                        nc.tensor.matmul(
                            Ap[:, 512 * h : 512 * h + 512], lhsT=identb[:],
                            rhs=n1p[:, 512 * h : 512 * h + 512], start=False, stop=True,
                        )
                    for h in range(2):
                        nc.tensor.matmul(
                            Bpair[:, 512 * h : 512 * h + 512], lhsT=negW2[:],
                            rhs=n1p[:, 512 * h : 512 * h + 512], start=True, stop=False,
                        )
                    # v1 = Q1 * (A + negd0) per half (STT: scalar slot carries
                    # the per-partition constant term of -W1^T n0)
                    for h in range(2):
                        r = r0 + h
                        cA = cA_D if N1_ENG[r % 16] == "D" else negd0T
                        nc.vector.scalar_tensor_tensor(
                            v1p[:, 512 * h : 512 * h + 512],
                            Ap[:, 512 * h : 512 * h + 512],
                            cA[:, r : r + 1],
                            Q1p[:, 512 * h : 512 * h + 512],
                            op0=OP.add,
                            op1=OP.mult,
                        )
                    for h in range(2):
                        nc.tensor.matmul(
                            Bpair[:, 512 * h : 512 * h + 512], lhsT=posW2[:],
                            rhs=v1p[:, 512 * h : 512 * h + 512], start=False, stop=False,
                        )
                    for h in range(2):
                        nc.tensor.matmul(
                            Bpair[:, 512 * h : 512 * h + 512], lhsT=identb[:],
                            rhs=n2p[:, 512 * h : 512 * h + 512], start=False, stop=True,
                        )
                    # v2 = Q2 * (B + cB) per half; cB carries the E1^2
                    # rank-1 terms omitted by DVE-produced n1/n2
                    for h in range(2):
                        r = r0 + h
                        k1 = N1_ENG[r % 16] == "D"
                        k2 = N2_ENG[r % 16] == "D"
                        cB = (cB_11 if k2 else cB_10) if k1 else (cB_01 if k2 else None)
                        sc = cB[:, r : r + 1] if cB is not None else 0.0
                        nc.vector.scalar_tensor_tensor(
                            v2p[:, 512 * h : 512 * h + 512],
                            Bpair[:, 512 * h : 512 * h + 512],
                            sc,
                            Q2p[:, 512 * h : 512 * h + 512],
                            op0=OP.add,
                            op1=OP.mult,
                        )
                    for h in range(2):
                        r = r0 + h
                        po = 32 * (r % 4)
                        nc.tensor.matmul(
                            C4[po : po + 1, :], lhsT=negonesb[:],
                            rhs=v2p[:, 512 * h : 512 * h + 512], start=False, stop=True,
                            tile_position=(0, po),
                        )
                    if rp % 2 == 1:
                        # C4 holds S2 - 1^T v2 = out rows; cheap fp32 copy to
                        # SBUF (DMA cannot read PSUM), then one DMA out
                        stag = rowp.tile([128, 512], F32, tag="stag")
                        nc.scalar.copy(stag[:], C4[:])
                        nc.sync.dma_start(
                            dout.ap()[r0 - 2 : r0 + 2, :], stag[:][0:97:32, :]
                        )
    nc.compile()
    return nc


def _get_nc():
    if "nc" not in _cache:
        _cache["nc"] = _build()
    return _cache["nc"]


def kernel(**inputs):
    nc = _get_nc()
    ident = np.eye(D, dtype=np.float32)
    in_maps = []
    for c in range(N_CORES):
        sl = slice(c * RPC, (c + 1) * RPC)
        m = {"ident": ident}
        for l in range(3):
            m[f"emb1_{l}"] = np.ascontiguousarray(inputs[f"emb1_{l}"][sl])
            m[f"emb2_{l}"] = np.asarray(inputs[f"emb2_{l}"])
        for l in (1, 2):
            m[f"cert1_{l}"] = np.ascontiguousarray(inputs[f"cert1_{l}"][sl])
            m[f"cert2_{l}"] = np.asarray(inputs[f"cert2_{l}"])
            m[f"alpha_{l}"] = np.asarray(inputs[f"alpha_{l}"]).reshape(D, 1)
            m[f"beta_{l}"] = np.asarray(inputs[f"beta_{l}"]).reshape(D, 1)
        for l in range(2):
            m[f"link_{l}"] = np.asarray(inputs[f"link_{l}"])
        in_maps.append(m)
    trace = bool(int(os.environ.get("AVSL_TRACE", "0")))
    res = run_bass_kernel_spmd(nc, in_maps, core_ids=list(range(N_CORES)), trace=trace)
    _cache["last_result"] = res
    return np.concatenate([res.results[c]["ovr"] for c in range(N_CORES)], axis=0)


# revision 12
# speedup vs baseline: 1.0984x; 1.0913x over previous
"""AVSL similarity kernel for Trainium2 (8 NeuronCores, data-parallel over B1).

Math (per (b1,b2) pair, d-vector chain over 3 layers):
  n_l = (normalize(emb1_l[b1]) - normalize(emb2_l[b2]))**2        [D]
  hat_0 = n_0
  hat_l = (1-P_l) * (hat_{l-1} @ W_l) + P_l * n_l,  l=1,2
  P_l   = sigmoid(alpha_l * cert1_l[b1] * cert2_l[b2] + beta_l)
  W_l   = col-top3-masked, col-normalized link_{l-1}
  out[b1,b2] = sum_d hat_2

Device decomposition, [d(=128 partitions), b2(=512 free)] layout, Q_l = 1-P_l
(sigmoid of negated argument):
  A  = n1 - W1^T n0                       (PE: negV0 matmul + I*negE0 + I*n1)
  v1 = Q1 * A          => hat1 = n1 - v1  (DVE STT; Q via ACT)
  B  = n2 - W2^T n1 + W2^T v1             (PE)
  v2 = Q2 * B          => hat2 = n2 - v2  (DVE)
  out_row = (2 - 2*E1n2.E2n2^T)[r,:] - 1^T v2
            (closed-form sum_d n2; S2 rows land in C4 via a PE identity
             matmul, colsum matmuls accumulate -1^T v2 on top, and the
             result is DMA'd to DRAM directly from PSUM.)
Rows processed in pairs; n1/n2/Q tiles span the pair ([128,1024]) so the
shared-stationary matmuls run at FD=1024 (halves the PE instruction count).
Engine split for the n1/n2 squares is tunable per (r%16) slot across
ACT (Square activation), DVE (bf16 TS add + TT mul) and GPSIMD.
Matmul operands bf16.

Sharding: emb1/cert1 rows split 64/core; emb2/cert2/links/alpha/beta replicated.
"""
import os
import sys

sys.path.insert(0, "/opt/trn_rl_repo")

import numpy as np

import concourse.bass as bass
import concourse.tile as tile
from concourse import bacc, mybir
from concourse.bass_utils import run_bass_kernel_spmd

N_CORES = 8
B1, B2, D = 512, 512, 128
RPC = B1 // N_CORES  # rows of ovr_sim per core
F32 = mybir.dt.float32
BF16 = mybir.dt.bfloat16
AF = mybir.ActivationFunctionType
OP = mybir.AluOpType
AX = mybir.AxisListType

# per (r % 16) engine assignment for the n1/n2 elementwise passes:
# 'A' = ACT (Square activation, includes the E1^2 rank-1 term),
# 'D' = DVE (single STT: n = E2^2 - 2*E1*E2; the missing E1^2 term is
#       folded into the v1/v2 STT scalar columns, chosen per slot).
# GPSIMD was tried and is a net loss: SBUF port contention slows DVE
# ~20-25% and its semaphore handling costs ~0.9us per sync.
N1_ENG = ['A' if s % 2 == 0 else 'D' for s in range(16)]
N2_ENG = ['A' if (s % 2 == 1 and s != 15) else 'D' for s in range(16)]


def _bcast(ap_col, cols):
    return ap_col.broadcast_to((128, cols))

_cache = {}


def _norm_blocks(nc, pre, t, tag_prefix, parts):
    """l2-normalize rows of an SBUF tile [parts,128]; returns normalized tile."""
    sq = pre.tile([parts, 128], F32, tag=f"{tag_prefix}sq")
    nc.vector.tensor_mul(sq[:], t[:], t[:])
    ss = pre.tile([parts, 1], F32, tag=f"{tag_prefix}ss")
    nc.vector.reduce_sum(ss[:], sq[:], axis=AX.X)
    nrm = pre.tile([parts, 1], F32, tag=f"{tag_prefix}nrm")
    nc.scalar.sqrt(nrm[:], ss[:])
    nrm2 = pre.tile([parts, 1], F32, tag=f"{tag_prefix}nrm2")
    nc.vector.tensor_scalar_max(nrm2[:], nrm[:], 1e-12)
    rn = pre.tile([parts, 1], F32, tag=f"{tag_prefix}rn")
    nc.vector.reciprocal(rn[:], nrm2[:])
    tn = pre.tile([parts, 128], F32, tag=f"{tag_prefix}tn")
    nc.vector.tensor_scalar_mul(tn[:], t[:], rn[:])
    return tn


def _transpose_512(nc, pre, pps_tile, ident, dram_ap, normalize):
    """Load [512,128] DRAM tensor, optionally l2-normalize rows, transpose
    into the given PSUM tile [128,512]."""
    for blk in range(4):
        t = pre.tile([128, 128], F32, tag="ld")
        nc.sync.dma_start(t[:], dram_ap[blk * 128 : (blk + 1) * 128, :])
        if normalize:
            t = _norm_blocks(nc, pre, t, "n", 128)
        nc.tensor.transpose(pps_tile[:, blk * 128 : (blk + 1) * 128], t[:], ident[:])


def _prep_link(nc, pre, pps, const, ident, dram_ap, i, want_pos, want_f32=False):
    """Top-3-per-column mask + column-normalize of link [d,e].
    Returns (negW bf16 [d,e], W bf16 [d,e] or None, negW fp32 or None)."""
    lt = pre.tile([128, 128], F32, tag="wld")
    nc.sync.dma_start(lt[:], dram_ap[:, :])
    tpw = pps.tile([128, 128], F32, tag="tpw")
    nc.tensor.transpose(tpw[:], lt[:], ident[:])
    wt = pre.tile([128, 128], F32, tag="wt")
    nc.scalar.copy(wt[:], tpw[:])  # [e, d]

    x = wt
    m = None
    for k in range(3):
        m = pre.tile([128, 1], F32, tag=f"wm{k}")
        nc.vector.reduce_max(m[:], x[:], axis=AX.X)
        if k < 2:
            msk = pre.tile([128, 128], F32, tag=f"wmask{k}")
            # ((x >= m) * -2) + x : push current max below everything
            nc.vector.tensor_scalar(msk[:], x[:], m[:], -2.0, op0=OP.is_ge, op1=OP.mult)
            x2 = pre.tile([128, 128], F32, tag=f"wx{k}")
            nc.vector.tensor_add(x2[:], x[:], msk[:])
            x = x2
    # m = 3rd-largest original value per row; keep entries >= m
    wm = pre.tile([128, 128], F32, tag="wkeep")
    nc.vector.scalar_tensor_tensor(wm[:], wt[:], m[:], wt[:], op0=OP.is_ge, op1=OP.mult)
    cs = pre.tile([128, 1], F32, tag="wcs")
    nc.vector.reduce_sum(cs[:], wm[:], axis=AX.X)
    cse = pre.tile([128, 1], F32, tag="wcse")
    nc.vector.tensor_scalar_add(cse[:], cs[:], 1e-8)
    rc = pre.tile([128, 1], F32, tag="wrc")
    nc.vector.reciprocal(rc[:], cse[:])
    nrc = pre.tile([128, 1], F32, tag="wnrc")
    nc.scalar.mul(nrc[:], rc[:], -1.0)
    wnT = pre.tile([128, 128], F32, tag=f"wnT{i}", name=f"wnT{i}")
    nc.vector.tensor_scalar_mul(wnT[:], wm[:], nrc[:])  # [e, d] (negated)
    tpw2 = pps.tile([128, 128], F32, tag="tpw")
    nc.tensor.transpose(tpw2[:], wnT[:], ident[:])
    negw = const.tile([128, 128], BF16, tag=f"negW{i}", name=f"negW{i}")
    nc.scalar.copy(negw[:], tpw2[:])  # [d, e] bf16, negated
    posw = None
    if want_pos:
        posw = const.tile([128, 128], BF16, tag=f"posW{i}", name=f"posW{i}")
        nc.scalar.mul(posw[:], tpw2[:], -1.0)  # [d, e] bf16, positive
    negwf = None
    if want_f32:
        negwf = const.tile([128, 128], F32, tag=f"negWf{i}", name=f"negWf{i}")
        nc.scalar.copy(negwf[:], tpw2[:])  # [d, e] fp32, negated
    return negw, posw, negwf


def _build():
    nc = bacc.Bacc("TRN2", target_bir_lowering=False, debug=False)
    de1 = [nc.dram_tensor(f"emb1_{l}", [RPC, D], F32, kind="ExternalInput") for l in range(3)]
    dc1 = [nc.dram_tensor(f"cert1_{l}", [RPC, D], F32, kind="ExternalInput") for l in (1, 2)]
    de2 = [nc.dram_tensor(f"emb2_{l}", [B2, D], F32, kind="ExternalInput") for l in range(3)]
    dc2 = [nc.dram_tensor(f"cert2_{l}", [B2, D], F32, kind="ExternalInput") for l in (1, 2)]
    dal = [nc.dram_tensor(f"alpha_{l}", [D, 1], F32, kind="ExternalInput") for l in (1, 2)]
    dbe = [nc.dram_tensor(f"beta_{l}", [D, 1], F32, kind="ExternalInput") for l in (1, 2)]
    dlk = [nc.dram_tensor(f"link_{l}", [D, D], F32, kind="ExternalInput") for l in range(2)]
    did = nc.dram_tensor("ident", [D, D], F32, kind="ExternalInput")
    dout = nc.dram_tensor("ovr", [RPC, B2], F32, kind="ExternalOutput")

    with tile.TileContext(nc) as tc:
        with tc.tile_pool(name="const", bufs=1) as const:
            ident = const.tile([128, 128], F32, tag="ident")
            nc.sync.dma_start(ident[:], did.ap())
            identb = const.tile([128, 128], BF16, tag="identb")
            nc.vector.tensor_copy(identb[:], ident[:])
            onesb = const.tile([128, 1], BF16, tag="onesb")
            nc.vector.memset(onesb[:], 1.0)
            negonesb = const.tile([128, 1], BF16, tag="negonesb")
            nc.vector.memset(negonesb[:], -1.0)
            nacol = []
            nbcol = []
            for i in range(2):
                a = const.tile([128, 1], F32, tag=f"acol{i}", name=f"acol{i}")
                nc.sync.dma_start(a[:], dal[i].ap())
                na = const.tile([128, 1], F32, tag=f"nacol{i}", name=f"nacol{i}")
                nc.scalar.mul(na[:], a[:], -1.0)
                nacol.append(na)
                b = const.tile([128, 1], F32, tag=f"bcol{i}", name=f"bcol{i}")
                nc.sync.dma_start(b[:], dbe[i].ap())
                nb = const.tile([128, 1], F32, tag=f"nbcol{i}", name=f"nbcol{i}")
                nc.scalar.mul(nb[:], b[:], -1.0)
                nbcol.append(nb)

            # e2T: l=1/2 fp32 (ACT in) + bf16 (DVE/GPS in); l=0 bf16 (matmul rhs)
            e2T = [None] * 3
            e2T1b = None
            e2T2b = None
            c2T = [None] * 2
            e1T = [None] * 3  # l=0,2: positive; l=1: negated (ACT bias)
            ne1T2 = None  # negated l=2 (ACT bias for ACT-rows)
            nscT = [None] * 2
            with tc.tile_pool(name="pre", bufs=4) as pre, tc.tile_pool(
                name="prepsum", bufs=2, space="PSUM"
            ) as pps:
                for l in range(3):
                    tp = pps.tile([128, 512], F32, tag="tp512")
                    _transpose_512(nc, pre, tp, ident, de2[l].ap(), True)
                    dt = F32 if l in (1, 2) else BF16
                    e2T[l] = const.tile([128, 512], dt, tag=f"e2T{l}", name=f"e2T{l}")
                    nc.scalar.copy(e2T[l][:], tp[:])
                    if l == 1:
                        e2T1b = const.tile([128, 512], BF16, tag="e2T1b")
                        nc.vector.tensor_copy(e2T1b[:], tp[:])
                    if l == 2:
                        e2T2b = const.tile([128, 512], BF16, tag="e2T2b")
                        nc.vector.tensor_copy(e2T2b[:], tp[:])
                for i in range(2):
                    tp = pps.tile([128, 512], F32, tag="tp512")
                    _transpose_512(nc, pre, tp, ident, dc2[i].ap(), False)
                    c2T[i] = const.tile([128, 512], BF16, tag=f"c2T{i}", name=f"c2T{i}")
                    nc.scalar.copy(c2T[i][:], tp[:])
                # emb1 shard: normalize rows, transpose -> [d, r]
                for l in range(3):
                    t = pre.tile([64, 128], F32, tag="e1ld")
                    nc.sync.dma_start(t[:], de1[l].ap())
                    tn = _norm_blocks(nc, pre, t, "e1", 64)
                    if l == 1:
                        tn2 = pre.tile([64, 128], F32, tag="e1neg")
                        nc.scalar.mul(tn2[:], tn[:], -1.0)
                        tn = tn2
                    tp64 = pps.tile([128, 64], F32, tag="tp64")
                    nc.tensor.transpose(tp64[:], tn[:], ident[:64, :64])
                    e1T[l] = const.tile([128, 64], F32, tag=f"e1T{l}", name=f"e1T{l}")
                    nc.scalar.copy(e1T[l][:], tp64[:])
                    if l == 2:
                        ne1T2 = const.tile([128, 64], F32, tag="ne1T2")
                        nc.scalar.mul(ne1T2[:], tp64[:], -1.0)
                        m2e1T2b = const.tile([128, 64], BF16, tag="m2e1T2b")
                        nc.scalar.mul(m2e1T2b[:], tp64[:], -2.0)
                # cert1 shard: transpose, scale by -alpha -> [d, r]
                for i in range(2):
                    t = pre.tile([64, 128], F32, tag="c1ld")
                    nc.sync.dma_start(t[:], dc1[i].ap())
                    tp64 = pps.tile([128, 64], F32, tag="tp64")
                    nc.tensor.transpose(tp64[:], t[:], ident[:64, :64])
                    c1T = pre.tile([128, 64], F32, tag="c1T")
                    nc.scalar.copy(c1T[:], tp64[:])
                    nscT[i] = const.tile([128, 64], F32, tag=f"nscT{i}", name=f"nscT{i}")
                    nc.vector.tensor_scalar_mul(nscT[i][:], c1T[:], nacol[i][:])
                negW1, _, negW1f = _prep_link(nc, pre, pps, const, ident, dlk[0].ap(), 0, False, True)
                negW2, posW2, negW2f = _prep_link(nc, pre, pps, const, ident, dlk[1].ap(), 1, True, True)
                # constants for virtualized n0: n0 = e2sq0 + a0*e2T0 + c0
                e2sqT0 = pre.tile([128, 512], F32, tag="e2sqT0", name="e2sqT0")
                nc.vector.tensor_mul(e2sqT0[:], e2T[0][:], e2T[0][:])
                tpE = pps.tile([128, 512], F32, tag="tp512")
                nc.tensor.matmul(tpE[:], lhsT=negW1f[:], rhs=e2sqT0[:], start=True, stop=True)
                negE0 = const.tile([128, 512], BF16, tag="negE0", name="negE0")
                nc.scalar.copy(negE0[:], tpE[:])
                a0T = const.tile([128, 64], F32, tag="a0T", name="a0T")
                nc.scalar.mul(a0T[:], e1T[0][:], -2.0)
                c0T = pre.tile([128, 64], F32, tag="c0T", name="c0T")
                nc.vector.tensor_mul(c0T[:], e1T[0][:], e1T[0][:])
                tpD = pps.tile([128, 64], F32, tag="tp64")
                nc.tensor.matmul(tpD[:], lhsT=negW1f[:], rhs=c0T[:], start=True, stop=True)
                negd0T = const.tile([128, 64], F32, tag="negd0T", name="negd0T")
                nc.scalar.copy(negd0T[:], tpD[:])
                # negd0 in rows-layout (bf16) + ones tile: the per-row
                # constant of -W1^T n0 enters Ap as a K=1 rank-1 matmul so
                # v1 becomes a single pair-wide TT
                tpN = pps.tile([64, 512], F32, tag="tpS")
                nc.tensor.transpose(tpN[:, 0:128], negd0T[:], ident[:])
                negd0rb = pre.tile([64, 128], BF16, tag="negd0rb")
                nc.scalar.copy(negd0rb[:], tpN[:, 0:128])
                # flatten to partition 0 (matmul lhsT base partition must be
                # 0/32/64): row r lives at free offset 128*r
                negd0fl = const.tile([1, 8192], BF16, tag="negd0fl")
                nc.sync.dma_start(negd0fl[:], negd0rb[:])
                ones1 = const.tile([1, 512], BF16, tag="ones1")
                nc.vector.memset(ones1[:], 1.0)
                # closed-form row-sum of n2: sum_d n2 = 2 - 2*E1n2.E2n2^T
                psS = pps.tile([64, 512], F32, tag="tpS")
                nc.tensor.matmul(psS[:], lhsT=m2e1T2b[:], rhs=e2T2b[:], start=True, stop=True)
                twos = const.tile([64, 1], F32, tag="twos")
                nc.vector.memset(twos[:], 2.0)
                S2sb = const.tile([64, 512], BF16, tag="S2sb", name="S2sb")
                nc.scalar.activation(S2sb[:], psS[:], AF.Identity, bias=twos[:])
                # rearrange S2 rows to the C4 partition layout (row 4g+k ->
                # partition 32k, free block g); zero unused partitions so the
                # identity matmul into C4 reads defined data everywhere
                S2str = const.tile([128, 8192], BF16, tag="S2str", name="S2str")
                nc.vector.memset(S2str[:], 0.0)
                for k in range(4):
                    nc.sync.dma_start(
                        S2str[:][32 * k : 32 * k + 1, :], S2sb[:][k:64:4, :]
                    )

            with tc.tile_pool(name="row", bufs=10) as rowp, tc.tile_pool(
                name="pair", bufs=4
            ) as pairp, tc.tile_pool(name="psA", bufs=2, space="PSUM") as psA, tc.tile_pool(
                name="psB", bufs=1, space="PSUM"
            ) as psB, tc.tile_pool(name="psC", bufs=2, space="PSUM") as psC:
                C4 = None
                for rp in range(RPC // 2):
                    r0 = 2 * rp
                    Ap = psA.tile([128, 1024], F32, tag="Ap")
                    Bpair = psB.tile([128, 1024], F32, tag="Bpair")
                    Q1p = pairp.tile([128, 1024], F32, tag="Q1p")
                    Q2p = pairp.tile([128, 1024], F32, tag="Q2p")
                    n1p = pairp.tile([128, 1024], BF16, tag="n1p")
                    n2p = pairp.tile([128, 1024], BF16, tag="n2p")
                    n0h = [None, None]
                    v1p = pairp.tile([128, 1024], BF16, tag="v1p")
                    v2p = pairp.tile([128, 1024], BF16, tag="v2p")
                    if rp % 2 == 0:
                        # fresh C4: seed with the S2 closed-form rows via PE
                        C4 = psC.tile([128, 512], F32, tag="C4")
                        g4 = rp // 2
                        nc.tensor.matmul(
                            C4[:, :], lhsT=identb[:],
                            rhs=S2str[:, 512 * g4 : 512 * g4 + 512],
                            start=True, stop=False,
                        )
                    for h in range(2):
                        r = r0 + h
                        fo = 512 * h
                        # n0 virtualized: negV0 = a0 * (-W1); its matmul against
                        # e2T0 plus I*negE0 reproduce -W1^T n0 up to the
                        # per-partition constant negd0T handled in the v1 STT
                        negV0 = rowp.tile([128, 128], BF16, tag="negV0")
                        nc.vector.tensor_scalar_mul(
                            negV0[:], negW1[:], a0T[:, r : r + 1]
                        )
                        n0h[h] = negV0
                        # n1 on ACT (Square activation) or DVE (TS add + TT mul)
                        if N1_ENG[r % 16] == "A":
                            nc.scalar.activation(
                                n1p[:, fo : fo + 512], e2T[1][:], AF.Square,
                                bias=e1T[1][:, r : r + 1],
                            )
                        else:
                            d1 = rowp.tile([128, 512], BF16, tag="d1")
                            nc.vector.tensor_scalar_add(
                                d1[:], e2T1b[:], e1T[1][:, r : r + 1]
                            )
                            nc.vector.tensor_mul(n1p[:, fo : fo + 512], d1[:], d1[:])
                        # n2 on ACT or DVE likewise
                        if N2_ENG[r % 16] == "A":
                            nc.scalar.activation(
                                n2p[:, fo : fo + 512], e2T[2][:], AF.Square,
                                bias=ne1T2[:, r : r + 1],
                            )
                        else:
                            d2 = rowp.tile([128, 512], BF16, tag="d2")
                            nc.vector.tensor_scalar_add(
                                d2[:], e2T2b[:], ne1T2[:, r : r + 1]
                            )
                            nc.vector.tensor_mul(n2p[:, fo : fo + 512], d2[:], d2[:])
                        nc.scalar.activation(
                            Q1p[:, fo : fo + 512],
                            c2T[0][:],
                            AF.Sigmoid,
                            bias=nbcol[0][:],
                            scale=nscT[0][:, r : r + 1],
                        )
                        nc.scalar.activation(
                            Q2p[:, fo : fo + 512],
                            c2T[1][:],
                            AF.Sigmoid,
                            bias=nbcol[1][:],
                            scale=nscT[1][:, r : r + 1],
                        )
                    # matmuls interleaved across the two halves so consecutive
                    # PE ops never accumulate into the same PSUM region
                    for h in range(2):
                        nc.tensor.matmul(
                            Ap[:, 512 * h : 512 * h + 512], lhsT=n0h[h][:],
                            rhs=e2T[0][:], start=True, stop=False,
                        )
                    for h in range(2):
                        nc.tensor.matmul(
                            Ap[:, 512 * h : 512 * h + 512], lhsT=identb[:],
                            rhs=negE0[:], start=False, stop=False,
                        )
                    for h in range(2):
                        nc.tensor.matmul(
                            Ap[:, 512 * h : 512 * h + 512], lhsT=identb[:],
                            rhs=n1p[:, 512 * h : 512 * h + 512], start=False, stop=False,
                        )
                    for h in range(2):
                        r = r0 + h
                        nc.tensor.matmul(
                            Ap[:, 512 * h : 512 * h + 512],
                            lhsT=negd0fl[0:1, 128 * r : 128 * r + 128],
                            rhs=ones1[0:1, :], start=False, stop=True,
                        )
                    for h in range(2):
                        nc.tensor.matmul(
                            Bpair[:, 512 * h : 512 * h + 512], lhsT=negW2[:],
                            rhs=n1p[:, 512 * h : 512 * h + 512], start=True, stop=False,
                        )
                    # v1 = Q1 * A, one pair-wide TT (negd0 already in Ap)
                    nc.vector.tensor_mul(v1p[:], Q1p[:], Ap[:])
                    for h in range(2):
                        nc.tensor.matmul(
                            Bpair[:, 512 * h : 512 * h + 512], lhsT=posW2[:],
                            rhs=v1p[:, 512 * h : 512 * h + 512], start=False, stop=False,
                        )
                    for h in range(2):
                        nc.tensor.matmul(
                            Bpair[:, 512 * h : 512 * h + 512], lhsT=identb[:],
                            rhs=n2p[:, 512 * h : 512 * h + 512], start=False, stop=True,
                        )
                    # batched v2 = Q2 * B over the pair
                    nc.vector.tensor_mul(v2p[:], Q2p[:], Bpair[:])
                    for h in range(2):
                        r = r0 + h
                        po = 32 * (r % 4)
                        nc.tensor.matmul(
                            C4[po : po + 1, :], lhsT=negonesb[:],
                            rhs=v2p[:, 512 * h : 512 * h + 512], start=False, stop=True,
                            tile_position=(0, po),
                        )
                    if rp % 2 == 1:
                        # C4 holds S2 - 1^T v2 = out rows; cheap fp32 copy to
                        # SBUF (DMA cannot read PSUM), then one DMA out
                        stag = rowp.tile([128, 512], F32, tag="stag")
                        nc.scalar.copy(stag[:], C4[:])
                        nc.sync.dma_start(
                            dout.ap()[r0 - 2 : r0 + 2, :], stag[:][0:97:32, :]
                        )
    nc.compile()
    return nc


def _get_nc():
    if "nc" not in _cache:
        _cache["nc"] = _build()
    return _cache["nc"]


def kernel(**inputs):
    nc = _get_nc()
    ident = np.eye(D, dtype=np.float32)
    in_maps = []
    for c in range(N_CORES):
        sl = slice(c * RPC, (c + 1) * RPC)
        m = {"ident": ident}
        for l in range(3):
            m[f"emb1_{l}"] = np.ascontiguousarray(inputs[f"emb1_{l}"][sl])
            m[f"emb2_{l}"] = np.asarray(inputs[f"emb2_{l}"])
        for l in (1, 2):
            m[f"cert1_{l}"] = np.ascontiguousarray(inputs[f"cert1_{l}"][sl])
            m[f"cert2_{l}"] = np.asarray(inputs[f"cert2_{l}"])
            m[f"alpha_{l}"] = np.asarray(inputs[f"alpha_{l}"]).reshape(D, 1)
            m[f"beta_{l}"] = np.asarray(inputs[f"beta_{l}"]).reshape(D, 1)
        for l in range(2):
            m[f"link_{l}"] = np.asarray(inputs[f"link_{l}"])
        in_maps.append(m)
    trace = bool(int(os.environ.get("AVSL_TRACE", "0")))
    res = run_bass_kernel_spmd(nc, in_maps, core_ids=list(range(N_CORES)), trace=trace)
    _cache["last_result"] = res
    return np.concatenate([res.results[c]["ovr"] for c in range(N_CORES)], axis=0)
